# revision 30
# baseline (speedup 1.0000x reference)
"""Block-diagonal (per-frame) multi-head attention on 8 Trainium2 cores.

Problem: x[2,3200,512] -> QKV proj (H=8 heads, D=64) -> attention masked to
25-token frames (128 frames) -> out[2,3200,512].  N = 3200 = 128*25.

Sharding: 256 (batch, frame) groups; core c handles batch c//4, frames
(c%4)*32..+32  => 800 tokens/core, tiled as 8 x 100 tokens (4 frames).

Design (all-f16 matmuls, fp32 psum), v3:
  - Input DMA rides ONE hw queue (few BIG descriptors -- descriptor
    issue costs ~650ns each and sub-2KB rows lose ~25% aggregate DMA
    bandwidth): masks -> wq -> x-ch0 -> wk -> wv -> bvb -> x-ch1.
    x is packed chunk-contiguous (3200B rows), weights k-major (4KB).
  - The HAM clock ramps to 8/8 only after ~3.5us of GAPLESS PE
    activity, and any ~0.5us+ PE gap re-throttles it to 4/8 for ~4us,
    so the whole lead-in is bridged: NWARM junk matmuls cover the DMA
    doorbell latency (~1.5-2us) + first-arrival jitter, and dep-gated
    junk pads (reading just-arrived tiles so the scheduler cannot
    hoist them) bridge drain round-trips and inter-descriptor waits.
  - Emission follows DMA arrival: q-ch0, k-ch0, v0..v1, att0, ... with
    q/k ch0 projections as two-group passes accumulated k-slice by
    k-slice.  Projections contract over the partition dim.  Proj
    drains alternate Activation (Identity + per-partition bias) / DVE,
    halving the 2-deep acc psum ring's recycle latency; v-relu rides
    Activation, keeping every engine under the PE's busy window.
  - Per 100-token tile, scores live in TWO psum banks: stE [100, 4*100]
    holds the 4 even heads (PE rows 0-63), stO the odd heads (rows
    64-127).  A rank-5 mask matmul (f16-safe +-30000) initializes each
    bank; exp is ONE activation per bank.
  - PV output is split by head PARITY: pvE (heads 0,2,4,6 <- etE only)
    and pvO (odd <- etO only), each a single 1040B psum bank, 2-deep
    ring -- each PV half waits on only one exp, and the tail tiles
    borrow idle proj-accumulator banks so nothing serializes.
  - v has a ones-column per head so PV's last column yields the softmax
    denominator; per half-tile one reciprocal + one stride-0-broadcast
    multiply produce the normalized f16 output (host casts to f32).
"""

import numpy as np

B, N, DIN = 2, 3200, 512
H, D = 8, 64
TL, JN = 128, 25
NCORES = 8
TOK = 800      # tokens per core
NT = 8         # token tiles per core
TT = 100       # tokens per tile (4 frames)
CH = 400       # proj column-chunk (2 chunks)
NEGB = 30000.0  # additive mask magnitude (f16-safe; |scores| <~ 10)

# matmul dtype per stage: 'f32' | 'f32r' | 'bf16' | 'f16'
CONFIG = {"proj": "f16", "qk": "f16", "pv": "f16"}
FP8MASK = False  # fp8e5m2 DoubleRow mask matmuls (exact at +-2^15)
NWARM = 54     # PE-warmup filler matmuls during the input-DMA lead-in

_CACHE = {}
LAST_RESULT = None  # BassKernelResults of the most recent kernel() call


def _build(cfg):
    import concourse.bacc as bacc
    import concourse.tile as tile
    from concourse import mybir
    from concourse.bass import broadcast_tensor_aps

    f32 = mybir.dt.float32
    bf16 = mybir.dt.bfloat16
    f16 = mybir.dt.float16
    f32r = mybir.dt.float32r
    AF = mybir.ActivationFunctionType
    ALU = mybir.AluOpType

    def io_dt(kind):
        return {"f32": f32, "f32r": f32r, "bf16": bf16, "f16": f16}[kind]

    proj_dt = io_dt(cfg["proj"])
    qk_dt = io_dt(cfg["qk"])
    pv_dt = io_dt(cfg["pv"])
    mask_dt = f16 if cfg["qk"] == "f16" else bf16

    nc = bacc.Bacc("TRN2", target_bir_lowering=False, debug=False,
                   num_devices=NCORES)

    # xTp: chunk-contiguous: col = ch*1600 + k*400 + tok  (3200B DMA rows)
    xt_d = nc.dram_tensor("xTp", [128, 4 * TOK], proj_dt,
                          kind="ExternalInput").ap()
    # weights k-major: col = k*512 + f  (1KB k-slice DMA rows)
    w_d = {}
    for nm in ("wq", "wk", "wv"):
        w_d[nm] = nc.dram_tensor(nm, [128, 4 * DIN], proj_dt,
                                 kind="ExternalInput").ap()
    bqc_d = nc.dram_tensor("bqc", [128, 4], f32, kind="ExternalInput").ap()
    bkc_d = nc.dram_tensor("bkc", [128, 4], f32, kind="ExternalInput").ap()
    bvr_d = nc.dram_tensor("bvr", [1, DIN], f32, kind="ExternalInput").ap()
    mc_d = nc.dram_tensor("mC", [5, 5 * TT], mask_dt,
                          kind="ExternalInput").ap()
    if FP8MASK:
        mc8_d = nc.dram_tensor("mC8", [5, 10 * TT], mybir.dt.float8e5,
                               kind="ExternalInput").ap()
    out_d = nc.dram_tensor("out", [TOK, DIN], f16,
                          kind="ExternalOutput").ap()

    with tile.TileContext(nc) as tc:
        with (
            tc.tile_pool(name="pp", bufs=1) as pp,
            tc.tile_pool(name="sp", bufs=4) as sp,
            tc.tile_pool(name="ps", bufs=2, space="PSUM") as ps,
        ):
            # ---- persistent tiles ----
            wq_all = pp.tile([128, 4 * DIN], proj_dt, name="wq_all",
                             tag="wq_all")
            wk_all = pp.tile([128, 4 * DIN], proj_dt, name="wk_all",
                             tag="wk_all")
            wv_all = pp.tile([128, 4 * DIN], proj_dt, name="wv_all",
                             tag="wv_all")
            xt_all = pp.tile([128, 4 * TOK], proj_dt, name="xt_all",
                             tag="xt_all")

            # weights k-major views: slice (ft, k) -> [128, 128]
            def wsl(wt, ft, k):
                return wt[:, k * DIN + ft * 128:(k * DIN + (ft + 1) * 128)]

            wv = [wv_all[:, k * DIN:(k + 1) * DIN] for k in range(4)]
            # x chunk views: (ch, k) -> [128, 400] (tokens ch*400..+400)
            def xsl(ch, k):
                return xt_all[:, ch * 1600 + k * CH:
                              ch * 1600 + (k + 1) * CH]

            bqc = pp.tile([128, 4], f32, name="bqc", tag="bqc")
            bkc = pp.tile([128, 4], f32, name="bkc", tag="bkc")
            bvb = pp.tile([128, DIN], f32, name="bvb", tag="bvb")
            mc = pp.tile([128, 5 * TT], mask_dt, name="mc", tag="mc")
            ma = mc[:, 0:TT]
            mb4 = mc[:, TT:5 * TT]
            if FP8MASK:
                # fp8 DoubleRow factors: [K=5, ktile=2, M|N] with the
                # second k-tile zeroed, so either hw pairing convention
                # yields mA.T @ mB exactly (+-2^15 is exact in e5m2)
                mc8 = pp.tile([128, 10 * TT], mybir.dt.float8e5,
                              name="mc8", tag="mc8")
                ma8 = mc8[:, 0:2 * TT].rearrange("p (j c) -> p j c", j=2)
                mb8 = mc8[:, 2 * TT:10 * TT].rearrange("p (j c) -> p j c",
                                                       j=2)

            qt = [pp.tile([128, TOK], qk_dt, name=f"qt{k}", tag=f"qt{k}")
                  for k in range(4)]
            kt_ = [pp.tile([128, TOK], qk_dt, name=f"kt{k}", tag=f"kt{k}")
                   for k in range(4)]
            # v with 65 columns per head: col h*65+64 is all-ones so the PV
            # matmul also produces the softmax denominator in its last column
            vt = [pp.tile([TT, H * (D + 1)], pv_dt, name=f"vt{t}",
                          tag=f"vt{t}") for t in range(NT)]
            ot = [pp.tile([TT, DIN], f16, name=f"ot{t}", tag=f"ot{t}")
                  for t in range(NT)]

            # ---- PE warm-up: junk matmuls ramp the HAM clock from t~0.
            junk = pp.tile([128, 256], qk_dt, name="junk", tag="junk")
            nc.vector.memset(junk[:], 0.0)
            wacc = ps.tile([TT, 260], f32, name="wacc", tag="pv", bufs=2)
            for i in range(NWARM):
                nc.tensor.matmul(wacc[:, 0:128], junk[:, 0:TT],
                                 junk[:, 0:128], start=True, stop=True,
                                 skip_group_check=True)

            # ---- input DMAs: ONE hw queue (sync) in dependency order.
            # Descriptor ISSUE costs ~650ns each on the issuing engine, so
            # keep the count low.  Mask factors ride first (tiny; unblock
            # hoisted mask matmuls which double as clock-ramp warmup);
            # wq-ft0 + x-ch0-k01 give the first projection group an early
            # start.  bvb rides the scalar HW queue; biases ride gpsimd.
            nc.sync.dma_start(out=mc[0:5, :], in_=mc_d)
            nc.sync.dma_start(out=mc[64:69, :], in_=mc_d)
            if FP8MASK:
                nc.gpsimd.dma_start(out=mc8[0:5, :], in_=mc8_d)
                nc.gpsimd.dma_start(out=mc8[64:69, :], in_=mc8_d)
            nc.sync.dma_start(out=wq_all, in_=w_d["wq"])
            nc.sync.dma_start(out=xt_all[:, 0:1600], in_=xt_d[:, 0:1600])
            nc.sync.dma_start(out=wk_all, in_=w_d["wk"])
            nc.sync.dma_start(out=wv_all, in_=w_d["wv"])
            # broadcast the v-bias row to 100 partitions straight from DRAM
            bv_src, _ = broadcast_tensor_aps(bvr_d, bvb[0:TT, :])
            nc.sync.dma_start(out=bvb[0:TT, :], in_=bv_src)
            nc.sync.dma_start(out=xt_all[:, 1600:3200],
                              in_=xt_d[:, 1600:3200])
            nc.gpsimd.dma_start(out=bqc, in_=bqc_d)
            nc.gpsimd.dma_start(out=bkc, in_=bkc_d)

            def pad(n, dep=None):
                # dep: SBUF region whose DMA gates these fillers, so the
                # scheduler can't hoist them to the front of the PE stream
                src_ = junk if dep is None else dep
                for _ in range(n):
                    nc.tensor.matmul(wacc[:, 0:128], src_[:, 0:TT],
                                     src_[:, 0:128], start=True, stop=True,
                                     skip_group_check=True)

            # ---- stage emitters ----
            def qk_group(wt, bc, dst, ft, ch, drain):
                csl = slice(ch * CH, (ch + 1) * CH)
                acc = ps.tile([128, CH], f32, name="acc", tag="acc", bufs=2)
                for k in range(4):
                    nc.tensor.matmul(acc[:], wsl(wt, ft, k), xsl(ch, k),
                                     start=(k == 0), stop=(k == 3))
                if drain == "act":
                    # psum->sbuf drain + per-partition bias on the (idle)
                    # Activation engine, keeping DVE under the PE window
                    nc.scalar.activation(dst[ft][:, csl], acc[:],
                                         AF.Identity, bias=bc[:, ft:ft + 1])
                else:
                    nc.vector.tensor_scalar_add(dst[ft][:, csl], acc[:],
                                                bc[:, ft:ft + 1])

            def v_tile(t):
                ch, off = t // 4, (t % 4) * TT
                acc = ps.tile([TT, DIN], f32, name="vacc", tag="vacc", bufs=1)
                for k in range(4):
                    nc.tensor.matmul(acc[:], xsl(ch, k)[:, off:off + TT],
                                     wv[k][:],
                                     start=(k == 0), stop=(k == 3))
                vv = vt[t].rearrange("p (h c) -> p h c", c=D + 1)
                av = acc.rearrange("p (h c) -> p h c", c=D)
                bv = bvb[:TT, :].rearrange("p (h c) -> p h c", c=D)
                nc.vector.scalar_tensor_tensor(vv[:, :, :D], av, 0.0, bv,
                                               op0=ALU.add, op1=ALU.add)
                nc.scalar.activation(vv[:, :, :D], vv[:, :, :D], AF.Relu)
                nc.vector.memset(vv[:, :, D:D + 1], 1.0)

            def att_tile(t, tagE="st", tagO="st", tagPE="pv", tagPO="pv"):
                tsl = slice(t * TT, (t + 1) * TT)
                BUFS = {"st": 3, "acc": 2, "vacc": 1, "pv": 2}
                # two banks: even heads (PE rows 0-63) / odd heads (64-127).
                stE = ps.tile([TT, 4 * TT], f32, name="stE", tag=tagE,
                              bufs=BUFS[tagE])
                stO = ps.tile([TT, 4 * TT], f32, name="stO", tag=tagO,
                              bufs=BUFS[tagO])
                if FP8MASK:
                    nc.tensor.matmul(stE[:], ma8[0:5], mb8[0:5],
                                     start=True, stop=False,
                                     perf_mode=mybir.MatmulPerfMode.DoubleRow,
                                     skip_group_check=True)
                    nc.tensor.matmul(stO[:], ma8[64:69], mb8[64:69],
                                     start=True, stop=False,
                                     perf_mode=mybir.MatmulPerfMode.DoubleRow,
                                     skip_group_check=True)
                else:
                    nc.tensor.matmul(stE[:], ma[0:5, :], mb4[0:5, :],
                                     start=True, stop=False,
                                     skip_group_check=True)
                    nc.tensor.matmul(stO[:], ma[64:69, :], mb4[64:69, :],
                                     start=True, stop=False,
                                     skip_group_check=True)
                for i in range(4):
                    c = slice(i * TT, (i + 1) * TT)
                    # head 2i: ft=i rows 0-63; head 2i+1: ft=i rows 64-127
                    nc.tensor.matmul(stE[:, c], kt_[i][0:64, tsl],
                                     qt[i][0:64, tsl],
                                     start=False, stop=(i == 3),
                                     skip_group_check=True)
                    nc.tensor.matmul(stO[:, c], kt_[i][64:128, tsl],
                                     qt[i][64:128, tsl],
                                     start=False, stop=(i == 3),
                                     skip_group_check=True)
                etE = sp.tile([TT, 4 * TT], pv_dt, name="etE", tag="et",
                              bufs=4)
                etO = sp.tile([TT, 4 * TT], pv_dt, name="etO", tag="et",
                              bufs=4)
                nc.scalar.activation(etE[:], stE[:], AF.Exp)
                nc.scalar.activation(etO[:], stO[:], AF.Exp)

                # PV by head parity: pvE <- etE only (heads 0,2,4,6),
                # pvO <- etO only -- each half waits on a single exp.
                # Each is one 1040B psum bank, 2-deep ring (+ borrows).
                rc = sp.tile([TT, 8], f32, name="rc", tag="rc", bufs=4)
                rcv = rc.rearrange("p (h c) -> p h c", c=1)
                ov = ot[t].rearrange("p (h c) -> p h c", c=D)
                for par, et in ((0, etE), (1, etO)):
                    pv = ps.tile([TT, 4 * 65], f32, name=f"pv{par}",
                                 tag=(tagPE, tagPO)[par],
                                 bufs=BUFS[(tagPE, tagPO)[par]])
                    for j in range(4):
                        nc.tensor.matmul(pv[:, j * 65:(j + 1) * 65],
                                         et[:, j * TT:(j + 1) * TT],
                                         vt[t][:, (2 * j + par) * 65:
                                                (2 * j + par + 1) * 65],
                                         start=True, stop=True,
                                         skip_group_check=True)
                    # normalize this parity half right away; heads sit at
                    # ot cols (2j+par)*64 (stride-2 head view)
                    pvq = pv.rearrange("p (h c) -> p h c", c=65)
                    rch = rcv[:, par::2]
                    nc.vector.reciprocal(rch, pvq[:, :, D:D + 1])
                    i0, i1 = broadcast_tensor_aps(pvq[:, :, 0:D], rch)
                    nc.vector.tensor_tensor(ov[:, par::2], i0, i1,
                                            op=ALU.mult)
                nc.sync.dma_start(out=out_d[tsl, :], in_=ot[t][:])

            # ---- pipelined emission (matches DMA arrival order); drains
            # alternate Act/DVE so the 2-deep acc psum ring recycles fast
            def dr(ft):
                return "act" if ft % 2 == 0 else "dve"

            def qk_pass(wt, bc, dst, fts, ch, dr0, npad=0, npad0=0):
                # two ft-groups accumulated k-by-k: each (w,x) k-slice
                # delivery unlocks 2 ap-400 matmuls -> sub-us PE waits
                # even when DMA arrival drifts.  npad junk matmuls after
                # each quantum keep the HAM clock alive while the next
                # k-slice is still in flight (gated on the current slice
                # so the scheduler cannot hoist them to the front).
                csl = slice(ch * CH, (ch + 1) * CH)
                accs = [(ft, ps.tile([128, CH], f32, name="acc",
                                     tag="acc", bufs=2)) for ft in fts]
                for k in range(4):
                    for ft, acc in accs:
                        nc.tensor.matmul(acc[:], wsl(wt, ft, k), xsl(ch, k),
                                         start=(k == 0), stop=(k == 3))
                    if npad and k < 3:
                        pad(npad, xsl(ch, k))
                    if npad0 and k == 0:
                        pad(npad0, xsl(ch, 0))
                for i, (ft, acc) in enumerate(accs):
                    if dr(dr0 + i) == "act":
                        nc.scalar.activation(dst[ft][:, csl], acc[:],
                                             AF.Identity,
                                             bias=bc[:, ft:ft + 1])
                    else:
                        nc.vector.tensor_scalar_add(dst[ft][:, csl], acc[:],
                                                    bc[:, ft:ft + 1])

            qk_pass(wq_all, bqc, qt, (0, 1), 0, 0)
            pad(5, xsl(0, 3))       # bridge the drain round-trip
            qk_pass(wq_all, bqc, qt, (2, 3), 0, 0)
            pad(9, wq_all[:, 3 * DIN:4 * DIN])
            qk_pass(wk_all, bkc, kt_, (0, 1), 0, 1)
            pad(5, wk_all[:, 3 * DIN:4 * DIN])
            qk_pass(wk_all, bkc, kt_, (2, 3), 0, 1)
            pad(8, wk_all[:, 3 * DIN:4 * DIN])   # bridge until wv lands
            v_tile(0)
            v_tile(1)
            att_tile(0)
            v_tile(2)
            v_tile(3)
            att_tile(1)
            for ft in range(4):
                qk_group(wq_all, bqc, qt, ft, 1, dr(ft))
            att_tile(2)
            for ft in range(4):
                qk_group(wk_all, bkc, kt_, ft, 1, dr(ft + 1))
            v_tile(4)
            att_tile(3)
            v_tile(5)
            att_tile(4)
            v_tile(6)
            att_tile(5, "acc", "acc")
            v_tile(7)
            att_tile(6, "st", "st")
            att_tile(7, "st", "st", "acc", "acc")

    nc.compile()
    return nc


def _prep_inputs(x, Wq, bq, Wk, bk, Wv, bv, cfg):
    import ml_dtypes

    x = np.asarray(x, np.float32)
    Wq = np.asarray(Wq, np.float32)
    bq = np.asarray(bq, np.float32)
    Wk = np.asarray(Wk, np.float32)
    bk = np.asarray(bk, np.float32)
    Wv = np.asarray(Wv, np.float32)
    bv = np.asarray(bv, np.float32)

    scale = 1.0 / np.sqrt(np.float32(D))  # 1/8, exact
    wq_s = (Wq * scale).astype(np.float32)
    bq_s = (bq * scale).astype(np.float32)

    io_np = {"bf16": ml_dtypes.bfloat16,
             "f16": np.float16}.get(cfg["proj"], np.float32)
    mask_np = np.float16 if cfg["qk"] == "f16" else ml_dtypes.bfloat16
    xT = np.ascontiguousarray(x.transpose(0, 2, 1))  # [B, DIN, N]

    bqc = np.ascontiguousarray(bq_s.reshape(4, 128).T)
    bkc = np.ascontiguousarray(bk.reshape(4, 128).T)
    bvr = np.ascontiguousarray(bv[None, :])

    # rank-5 factors of the additive frame mask over one 100-token tile
    big = mask_np(NEGB)
    mA = np.zeros((5, TT), mask_np)
    mB = np.zeros((5, TT), mask_np)
    mA[0, :] = 1
    mB[0, :] = -big
    for f in range(4):
        mA[1 + f, f * JN:(f + 1) * JN] = 1
        mB[1 + f, f * JN:(f + 1) * JN] = big
    mC = np.ascontiguousarray(
        np.concatenate([mA, np.tile(mB, (1, 4))], axis=1))
    f8 = ml_dtypes.float8_e5m2
    big8 = np.float32(32768.0)
    mC8 = np.zeros((5, 10 * TT), np.float32)
    mC8[:, 0:TT] = (mA != 0)                      # lhsT ktile0 = mA
    mb_f32 = np.tile(np.array(mB, np.float32), (1, 4))
    mb_f32 = np.where(mb_f32 > 0, big8, np.where(mb_f32 < 0, -big8, 0.0))
    mC8[:, 2 * TT:6 * TT] = mb_f32                # rhs ktile0 = mB4
    mC8 = mC8.astype(f8)



    def pack_w_k(w):
        # [512, 512] -> [128, 4*512]: col = k*512 + f  (4KB DMA rows)
        return np.ascontiguousarray(
            w.reshape(4, 128, DIN).transpose(1, 0, 2).reshape(128, 4 * DIN)
        ).astype(io_np)

    wq_p, wk_p, wv_p = pack_w_k(wq_s), pack_w_k(Wk), pack_w_k(Wv)

    in_maps = []
    for c in range(NCORES):
        b, fb = c // 4, c % 4
        xc = xT[b, :, fb * TOK:(fb + 1) * TOK]  # [512, 800]
        # chunk-contiguous: col = ch*1600 + k*400 + tok (3200B DMA rows)
        xt_p = np.ascontiguousarray(
            xc.reshape(4, 128, 2, CH).transpose(1, 2, 0, 3)
            .reshape(128, 4 * TOK)
        ).astype(io_np)
        in_maps.append({
            "xTp": xt_p,
            "wq": wq_p,
            "wk": wk_p,
            "wv": wv_p,
            "bqc": bqc, "bkc": bkc, "bvr": bvr, "mC": mC,
            **({"mC8": mC8} if FP8MASK else {}),
        })
    return in_maps


def kernel(x, Wq, bq, Wk, bk, Wv, bv, att_heads=H, latent_dim=D,
           time_len=TL, joint_num=JN, **_):
    from concourse.bass_utils import run_bass_kernel_spmd

    cfg = tuple(sorted(CONFIG.items()))
    if cfg not in _CACHE:
        _CACHE[cfg] = _build(CONFIG)
    nc = _CACHE[cfg]

    in_maps = _prep_inputs(x, Wq, bq, Wk, bk, Wv, bv, CONFIG)
    res = run_bass_kernel_spmd(nc, in_maps, core_ids=list(range(NCORES)))
    global LAST_RESULT
    LAST_RESULT = res

    out = np.empty((B, N, DIN), np.float32)
    for c in range(NCORES):
        b, fb = c // 4, c % 4
        out[b, fb * TOK:(fb + 1) * TOK, :] = res.results[c]["out"]
    return out


# revision 31
# speedup vs baseline: 1.0076x; 1.0076x over previous
"""Block-diagonal (per-frame) multi-head attention on 8 Trainium2 cores.

Problem: x[2,3200,512] -> QKV proj (H=8 heads, D=64) -> attention masked to
25-token frames (128 frames) -> out[2,3200,512].  N = 3200 = 128*25.

Sharding: 256 (batch, frame) groups; core c handles batch c//4, frames
(c%4)*32..+32  => 800 tokens/core, tiled as 8 x 100 tokens (4 frames).

Design (all-f16 matmuls, fp32 psum), v3:
  - Input DMA rides ONE hw queue (few BIG descriptors -- descriptor
    issue costs ~650ns each and sub-2KB rows lose ~25% aggregate DMA
    bandwidth): masks -> wq -> x-ch0 -> wk -> wv -> bvb -> x-ch1.
    x is packed chunk-contiguous (3200B rows), weights k-major (4KB).
  - The HAM clock ramps to 8/8 only after ~3.5us of GAPLESS PE
    activity, and any ~0.5us+ PE gap re-throttles it to 4/8 for ~4us,
    so the whole lead-in is bridged: NWARM junk matmuls cover the DMA
    doorbell latency (~1.5-2us) + first-arrival jitter, and dep-gated
    junk pads (reading just-arrived tiles so the scheduler cannot
    hoist them) bridge drain round-trips and inter-descriptor waits.
  - Emission follows DMA arrival: q-ch0, k-ch0, v0..v1, att0, ... with
    q/k ch0 projections as two-group passes accumulated k-slice by
    k-slice.  Projections contract over the partition dim.  Proj
    drains alternate Activation (Identity + per-partition bias) / DVE,
    halving the 2-deep acc psum ring's recycle latency; v-relu rides
    Activation, keeping every engine under the PE's busy window.
  - Per 100-token tile, scores live in TWO psum banks: stE [100, 4*100]
    holds the 4 even heads (PE rows 0-63), stO the odd heads (rows
    64-127).  A rank-5 mask matmul (f16-safe +-30000) initializes each
    bank; exp is ONE activation per bank.
  - PV output is split by head PARITY: pvE (heads 0,2,4,6 <- etE only)
    and pvO (odd <- etO only), each a single 1040B psum bank, 2-deep
    ring -- each PV half waits on only one exp, and the tail tiles
    borrow idle proj-accumulator banks so nothing serializes.
  - v has a ones-column per head so PV's last column yields the softmax
    denominator; per half-tile one reciprocal + one stride-0-broadcast
    multiply produce the normalized f16 output (host casts to f32).
"""

import numpy as np

B, N, DIN = 2, 3200, 512
H, D = 8, 64
TL, JN = 128, 25
NCORES = 8
TOK = 800      # tokens per core
NT = 8         # token tiles per core
TT = 100       # tokens per tile (4 frames)
CH = 400       # proj column-chunk (2 chunks)
NEGB = 30000.0  # additive mask magnitude (f16-safe; |scores| <~ 10)

# matmul dtype per stage: 'f32' | 'f32r' | 'bf16' | 'f16'
CONFIG = {"proj": "f16", "qk": "f16", "pv": "f16"}
FP8MASK = False  # fp8e5m2 DoubleRow mask matmuls (exact at +-2^15)
NWARM = 54     # PE-warmup filler matmuls during the input-DMA lead-in

_CACHE = {}
LAST_RESULT = None  # BassKernelResults of the most recent kernel() call


def _build(cfg):
    import concourse.bacc as bacc
    import concourse.tile as tile
    from concourse import mybir
    from concourse.bass import broadcast_tensor_aps

    f32 = mybir.dt.float32
    bf16 = mybir.dt.bfloat16
    f16 = mybir.dt.float16
    f32r = mybir.dt.float32r
    AF = mybir.ActivationFunctionType
    ALU = mybir.AluOpType

    def io_dt(kind):
        return {"f32": f32, "f32r": f32r, "bf16": bf16, "f16": f16}[kind]

    proj_dt = io_dt(cfg["proj"])
    qk_dt = io_dt(cfg["qk"])
    pv_dt = io_dt(cfg["pv"])
    mask_dt = f16 if cfg["qk"] == "f16" else bf16

    nc = bacc.Bacc("TRN2", target_bir_lowering=False, debug=False,
                   num_devices=NCORES)

    # xTp: chunk-contiguous: col = ch*1600 + k*400 + tok  (3200B DMA rows)
    xt_d = nc.dram_tensor("xTp", [128, 4 * TOK], proj_dt,
                          kind="ExternalInput").ap()
    # weights k-major: col = k*512 + f  (1KB k-slice DMA rows)
    w_d = {}
    for nm in ("wq", "wk", "wv"):
        w_d[nm] = nc.dram_tensor(nm, [128, 4 * DIN], proj_dt,
                                 kind="ExternalInput").ap()
    bqc_d = nc.dram_tensor("bqc", [128, 4], f32, kind="ExternalInput").ap()
    bkc_d = nc.dram_tensor("bkc", [128, 4], f32, kind="ExternalInput").ap()
    bvr_d = nc.dram_tensor("bvr", [1, DIN], f32, kind="ExternalInput").ap()
    mc_d = nc.dram_tensor("mC", [5, 5 * TT], mask_dt,
                          kind="ExternalInput").ap()
    if FP8MASK:
        mc8_d = nc.dram_tensor("mC8", [5, 10 * TT], mybir.dt.float8e5,
                               kind="ExternalInput").ap()
    out_d = nc.dram_tensor("out", [TOK, DIN], f16,
                          kind="ExternalOutput").ap()

    with tile.TileContext(nc) as tc:
        with (
            tc.tile_pool(name="pp", bufs=1) as pp,
            tc.tile_pool(name="sp", bufs=4) as sp,
            tc.tile_pool(name="ps", bufs=2, space="PSUM") as ps,
        ):
            # ---- persistent tiles ----
            wq_all = pp.tile([128, 4 * DIN], proj_dt, name="wq_all",
                             tag="wq_all")
            wk_all = pp.tile([128, 4 * DIN], proj_dt, name="wk_all",
                             tag="wk_all")
            wv_all = pp.tile([128, 4 * DIN], proj_dt, name="wv_all",
                             tag="wv_all")
            xt_all = pp.tile([128, 4 * TOK], proj_dt, name="xt_all",
                             tag="xt_all")

            # weights k-major views: slice (ft, k) -> [128, 128]
            def wsl(wt, ft, k):
                return wt[:, k * DIN + ft * 128:(k * DIN + (ft + 1) * 128)]

            wv = [wv_all[:, k * DIN:(k + 1) * DIN] for k in range(4)]
            # x chunk views: (ch, k) -> [128, 400] (tokens ch*400..+400)
            def xsl(ch, k):
                return xt_all[:, ch * 1600 + k * CH:
                              ch * 1600 + (k + 1) * CH]

            bqc = pp.tile([128, 4], f32, name="bqc", tag="bqc")
            bkc = pp.tile([128, 4], f32, name="bkc", tag="bkc")
            bvb = pp.tile([128, DIN], f32, name="bvb", tag="bvb")
            mc = pp.tile([128, 5 * TT], mask_dt, name="mc", tag="mc")
            ma = mc[:, 0:TT]
            mb4 = mc[:, TT:5 * TT]
            if FP8MASK:
                # fp8 DoubleRow factors: [K=5, ktile=2, M|N] with the
                # second k-tile zeroed, so either hw pairing convention
                # yields mA.T @ mB exactly (+-2^15 is exact in e5m2)
                mc8 = pp.tile([128, 10 * TT], mybir.dt.float8e5,
                              name="mc8", tag="mc8")
                ma8 = mc8[:, 0:2 * TT].rearrange("p (j c) -> p j c", j=2)
                mb8 = mc8[:, 2 * TT:10 * TT].rearrange("p (j c) -> p j c",
                                                       j=2)

            qt = [pp.tile([128, TOK], qk_dt, name=f"qt{k}", tag=f"qt{k}")
                  for k in range(4)]
            kt_ = [pp.tile([128, TOK], qk_dt, name=f"kt{k}", tag=f"kt{k}")
                   for k in range(4)]
            # v with 65 columns per head: col h*65+64 is all-ones so the PV
            # matmul also produces the softmax denominator in its last column
            vt = [pp.tile([TT, H * (D + 1)], pv_dt, name=f"vt{t}",
                          tag=f"vt{t}") for t in range(NT)]
            ot = [pp.tile([TT, DIN], f16, name=f"ot{t}", tag=f"ot{t}")
                  for t in range(NT)]

            # ---- PE warm-up: junk matmuls ramp the HAM clock from t~0.
            junk = pp.tile([128, 256], qk_dt, name="junk", tag="junk")
            nc.vector.memset(junk[:], 0.0)
            wacc = ps.tile([TT, 260], f32, name="wacc", tag="pv", bufs=2)
            for i in range(NWARM):
                nc.tensor.matmul(wacc[:, 0:128], junk[:, 0:TT],
                                 junk[:, 0:128], start=True, stop=True,
                                 skip_group_check=True)

            # ---- input DMAs: ONE hw queue (sync) in dependency order.
            # Descriptor ISSUE costs ~650ns each on the issuing engine, so
            # keep the count low.  Mask factors ride first (tiny; unblock
            # hoisted mask matmuls which double as clock-ramp warmup);
            # wq-ft0 + x-ch0-k01 give the first projection group an early
            # start.  bvb rides the scalar HW queue; biases ride gpsimd.
            nc.sync.dma_start(out=mc[0:5, :], in_=mc_d)
            nc.sync.dma_start(out=mc[64:69, :], in_=mc_d)
            if FP8MASK:
                nc.gpsimd.dma_start(out=mc8[0:5, :], in_=mc8_d)
                nc.gpsimd.dma_start(out=mc8[64:69, :], in_=mc8_d)
            nc.sync.dma_start(out=wq_all, in_=w_d["wq"])
            nc.sync.dma_start(out=xt_all[:, 0:1600], in_=xt_d[:, 0:1600])
            nc.sync.dma_start(out=wk_all, in_=w_d["wk"])
            nc.sync.dma_start(out=wv_all, in_=w_d["wv"])
            # broadcast the v-bias row to 100 partitions straight from DRAM
            bv_src, _ = broadcast_tensor_aps(bvr_d, bvb[0:TT, :])
            nc.sync.dma_start(out=bvb[0:TT, :], in_=bv_src)
            nc.sync.dma_start(out=xt_all[:, 1600:3200],
                              in_=xt_d[:, 1600:3200])
            nc.gpsimd.dma_start(out=bqc, in_=bqc_d)
            nc.gpsimd.dma_start(out=bkc, in_=bkc_d)

            def pad(n, dep=None):
                # dep: SBUF region whose DMA gates these fillers, so the
                # scheduler can't hoist them to the front of the PE stream
                src_ = junk if dep is None else dep
                for _ in range(n):
                    nc.tensor.matmul(wacc[:, 0:128], src_[:, 0:TT],
                                     src_[:, 0:128], start=True, stop=True,
                                     skip_group_check=True)

            # ---- stage emitters ----
            def qk_group(wt, bc, dst, ft, ch, drain):
                csl = slice(ch * CH, (ch + 1) * CH)
                acc = ps.tile([128, CH], f32, name="acc", tag="acc", bufs=2)
                for k in range(4):
                    nc.tensor.matmul(acc[:], wsl(wt, ft, k), xsl(ch, k),
                                     start=(k == 0), stop=(k == 3))
                if drain == "act":
                    # psum->sbuf drain + per-partition bias on the (idle)
                    # Activation engine, keeping DVE under the PE window
                    nc.scalar.activation(dst[ft][:, csl], acc[:],
                                         AF.Identity, bias=bc[:, ft:ft + 1])
                else:
                    nc.vector.tensor_scalar_add(dst[ft][:, csl], acc[:],
                                                bc[:, ft:ft + 1])

            def v_tile(t):
                ch, off = t // 4, (t % 4) * TT
                acc = ps.tile([TT, DIN], f32, name="vacc", tag="vacc", bufs=1)
                for k in range(4):
                    nc.tensor.matmul(acc[:], xsl(ch, k)[:, off:off + TT],
                                     wv[k][:],
                                     start=(k == 0), stop=(k == 3))
                vv = vt[t].rearrange("p (h c) -> p h c", c=D + 1)
                av = acc.rearrange("p (h c) -> p h c", c=D)
                bv = bvb[:TT, :].rearrange("p (h c) -> p h c", c=D)
                nc.vector.scalar_tensor_tensor(vv[:, :, :D], av, 0.0, bv,
                                               op0=ALU.add, op1=ALU.add)
                nc.scalar.activation(vv[:, :, :D], vv[:, :, :D], AF.Relu)
                nc.vector.memset(vv[:, :, D:D + 1], 1.0)

            BUFS = {"st": 3, "acc": 2, "vacc": 1, "pv": 2}

            def att_scores(t, tagE="st", tagO="st"):
                tsl = slice(t * TT, (t + 1) * TT)
                # two banks: even heads (PE rows 0-63) / odd heads (64-127).
                stE = ps.tile([TT, 4 * TT], f32, name="stE", tag=tagE,
                              bufs=BUFS[tagE])
                stO = ps.tile([TT, 4 * TT], f32, name="stO", tag=tagO,
                              bufs=BUFS[tagO])
                nc.tensor.matmul(stE[:], ma[0:5, :], mb4[0:5, :],
                                 start=True, stop=False,
                                 skip_group_check=True)
                nc.tensor.matmul(stO[:], ma[64:69, :], mb4[64:69, :],
                                 start=True, stop=False,
                                 skip_group_check=True)
                for i in range(4):
                    c = slice(i * TT, (i + 1) * TT)
                    # head 2i: ft=i rows 0-63; head 2i+1: ft=i rows 64-127
                    nc.tensor.matmul(stE[:, c], kt_[i][0:64, tsl],
                                     qt[i][0:64, tsl],
                                     start=False, stop=(i == 3),
                                     skip_group_check=True)
                    nc.tensor.matmul(stO[:, c], kt_[i][64:128, tsl],
                                     qt[i][64:128, tsl],
                                     start=False, stop=(i == 3),
                                     skip_group_check=True)
                etE = sp.tile([TT, 4 * TT], pv_dt, name="etE", tag="et",
                              bufs=4)
                etO = sp.tile([TT, 4 * TT], pv_dt, name="etO", tag="et",
                              bufs=4)
                nc.scalar.activation(etE[:], stE[:], AF.Exp)
                nc.scalar.activation(etO[:], stO[:], AF.Exp)
                return etE, etO

            def att_pv(t, ets, tagPE="pv", tagPO="pv"):
                # PV by head parity: pvE <- etE only (heads 0,2,4,6),
                # pvO <- etO only -- each half waits on a single exp.
                # Each is one 1040B psum bank, 2-deep ring (+ borrows).
                tsl = slice(t * TT, (t + 1) * TT)
                rc = sp.tile([TT, 8], f32, name="rc", tag="rc", bufs=4)
                rcv = rc.rearrange("p (h c) -> p h c", c=1)
                ov = ot[t].rearrange("p (h c) -> p h c", c=D)
                for par, et in ((0, ets[0]), (1, ets[1])):
                    pv = ps.tile([TT, 4 * 65], f32, name=f"pv{par}",
                                 tag=(tagPE, tagPO)[par],
                                 bufs=BUFS[(tagPE, tagPO)[par]])
                    for j in range(4):
                        nc.tensor.matmul(pv[:, j * 65:(j + 1) * 65],
                                         et[:, j * TT:(j + 1) * TT],
                                         vt[t][:, (2 * j + par) * 65:
                                                (2 * j + par + 1) * 65],
                                         start=True, stop=True,
                                         skip_group_check=True)
                    # normalize this parity half right away; heads sit at
                    # ot cols (2j+par)*64 (stride-2 head view)
                    pvq = pv.rearrange("p (h c) -> p h c", c=65)
                    rch = rcv[:, par::2]
                    nc.vector.reciprocal(rch, pvq[:, :, D:D + 1])
                    i0, i1 = broadcast_tensor_aps(pvq[:, :, 0:D], rch)
                    nc.vector.tensor_tensor(ov[:, par::2], i0, i1,
                                            op=ALU.mult)
                nc.sync.dma_start(out=out_d[tsl, :], in_=ot[t][:])

            def att_tile(t, tagE="st", tagO="st", tagPE="pv", tagPO="pv"):
                att_pv(t, att_scores(t, tagE, tagO), tagPE, tagPO)

            # ---- pipelined emission (matches DMA arrival order); drains
            # alternate Act/DVE so the 2-deep acc psum ring recycles fast
            def dr(ft):
                return "act" if ft % 2 == 0 else "dve"

            def qk_pass(wt, bc, dst, fts, ch, dr0, npad=0, npad0=0):
                # two ft-groups accumulated k-by-k: each (w,x) k-slice
                # delivery unlocks 2 ap-400 matmuls -> sub-us PE waits
                # even when DMA arrival drifts.  npad junk matmuls after
                # each quantum keep the HAM clock alive while the next
                # k-slice is still in flight (gated on the current slice
                # so the scheduler cannot hoist them to the front).
                csl = slice(ch * CH, (ch + 1) * CH)
                accs = [(ft, ps.tile([128, CH], f32, name="acc",
                                     tag="acc", bufs=2)) for ft in fts]
                for k in range(4):
                    for ft, acc in accs:
                        nc.tensor.matmul(acc[:], wsl(wt, ft, k), xsl(ch, k),
                                         start=(k == 0), stop=(k == 3))
                    if npad and k < 3:
                        pad(npad, xsl(ch, k))
                    if npad0 and k == 0:
                        pad(npad0, xsl(ch, 0))
                for i, (ft, acc) in enumerate(accs):
                    if dr(dr0 + i) == "act":
                        nc.scalar.activation(dst[ft][:, csl], acc[:],
                                             AF.Identity,
                                             bias=bc[:, ft:ft + 1])
                    else:
                        nc.vector.tensor_scalar_add(dst[ft][:, csl], acc[:],
                                                    bc[:, ft:ft + 1])

            qk_pass(wq_all, bqc, qt, (0, 1), 0, 0)
            pad(5, xsl(0, 3))       # bridge the drain round-trip
            qk_pass(wq_all, bqc, qt, (2, 3), 0, 0)
            pad(9, wq_all[:, 3 * DIN:4 * DIN])
            qk_pass(wk_all, bkc, kt_, (0, 1), 0, 1)
            pad(5, wk_all[:, 3 * DIN:4 * DIN])
            qk_pass(wk_all, bkc, kt_, (2, 3), 0, 1)
            pad(8, wk_all[:, 3 * DIN:4 * DIN])   # bridge until wv lands
            v_tile(0)
            v_tile(1)
            att_tile(0)
            v_tile(2)
            v_tile(3)
            att_tile(1)
            for ft in range(4):
                qk_group(wq_all, bqc, qt, ft, 1, dr(ft))
            att_tile(2)
            for ft in range(4):
                qk_group(wk_all, bkc, kt_, ft, 1, dr(ft + 1))
            v_tile(4)
            att_tile(3)
            v_tile(5)
            att_tile(4)
            v_tile(6)
            att_tile(5, "acc", "acc")
            v_tile(7)
            s6 = att_scores(6, "st", "st")
            s7 = att_scores(7, "st", "st")
            att_pv(6, s6)
            att_pv(7, s7, "acc", "acc")

    nc.compile()
    return nc


def _prep_inputs(x, Wq, bq, Wk, bk, Wv, bv, cfg):
    import ml_dtypes

    x = np.asarray(x, np.float32)
    Wq = np.asarray(Wq, np.float32)
    bq = np.asarray(bq, np.float32)
    Wk = np.asarray(Wk, np.float32)
    bk = np.asarray(bk, np.float32)
    Wv = np.asarray(Wv, np.float32)
    bv = np.asarray(bv, np.float32)

    scale = 1.0 / np.sqrt(np.float32(D))  # 1/8, exact
    wq_s = (Wq * scale).astype(np.float32)
    bq_s = (bq * scale).astype(np.float32)

    io_np = {"bf16": ml_dtypes.bfloat16,
             "f16": np.float16}.get(cfg["proj"], np.float32)
    mask_np = np.float16 if cfg["qk"] == "f16" else ml_dtypes.bfloat16
    xT = np.ascontiguousarray(x.transpose(0, 2, 1))  # [B, DIN, N]

    bqc = np.ascontiguousarray(bq_s.reshape(4, 128).T)
    bkc = np.ascontiguousarray(bk.reshape(4, 128).T)
    bvr = np.ascontiguousarray(bv[None, :])

    # rank-5 factors of the additive frame mask over one 100-token tile
    big = mask_np(NEGB)
    mA = np.zeros((5, TT), mask_np)
    mB = np.zeros((5, TT), mask_np)
    mA[0, :] = 1
    mB[0, :] = -big
    for f in range(4):
        mA[1 + f, f * JN:(f + 1) * JN] = 1
        mB[1 + f, f * JN:(f + 1) * JN] = big
    mC = np.ascontiguousarray(
        np.concatenate([mA, np.tile(mB, (1, 4))], axis=1))
    f8 = ml_dtypes.float8_e5m2
    big8 = np.float32(32768.0)
    mC8 = np.zeros((5, 10 * TT), np.float32)
    mC8[:, 0:TT] = (mA != 0)                      # lhsT ktile0 = mA
    mb_f32 = np.tile(np.array(mB, np.float32), (1, 4))
    mb_f32 = np.where(mb_f32 > 0, big8, np.where(mb_f32 < 0, -big8, 0.0))
    mC8[:, 2 * TT:6 * TT] = mb_f32                # rhs ktile0 = mB4
    mC8 = mC8.astype(f8)



    def pack_w_k(w):
        # [512, 512] -> [128, 4*512]: col = k*512 + f  (4KB DMA rows)
        return np.ascontiguousarray(
            w.reshape(4, 128, DIN).transpose(1, 0, 2).reshape(128, 4 * DIN)
        ).astype(io_np)

    wq_p, wk_p, wv_p = pack_w_k(wq_s), pack_w_k(Wk), pack_w_k(Wv)

    in_maps = []
    for c in range(NCORES):
        b, fb = c // 4, c % 4
        xc = xT[b, :, fb * TOK:(fb + 1) * TOK]  # [512, 800]
        # chunk-contiguous: col = ch*1600 + k*400 + tok (3200B DMA rows)
        xt_p = np.ascontiguousarray(
            xc.reshape(4, 128, 2, CH).transpose(1, 2, 0, 3)
            .reshape(128, 4 * TOK)
        ).astype(io_np)
        in_maps.append({
            "xTp": xt_p,
            "wq": wq_p,
            "wk": wk_p,
            "wv": wv_p,
            "bqc": bqc, "bkc": bkc, "bvr": bvr, "mC": mC,
            **({"mC8": mC8} if FP8MASK else {}),
        })
    return in_maps


def kernel(x, Wq, bq, Wk, bk, Wv, bv, att_heads=H, latent_dim=D,
           time_len=TL, joint_num=JN, **_):
    from concourse.bass_utils import run_bass_kernel_spmd

    cfg = tuple(sorted(CONFIG.items()))
    if cfg not in _CACHE:
        _CACHE[cfg] = _build(CONFIG)
    nc = _CACHE[cfg]

    in_maps = _prep_inputs(x, Wq, bq, Wk, bk, Wv, bv, CONFIG)
    res = run_bass_kernel_spmd(nc, in_maps, core_ids=list(range(NCORES)))
    global LAST_RESULT
    LAST_RESULT = res

    out = np.empty((B, N, DIN), np.float32)
    for c in range(NCORES):
        b, fb = c // 4, c % 4
        out[b, fb * TOK:(fb + 1) * TOK, :] = res.results[c]["out"]
    return out
